# revision 18
# baseline (speedup 1.0000x reference)
"""Multi-head attention (B=4, S=2048, C=768, H=8, HD=96) on 8 TRN2 NeuronCores.

Strategy: tensor-parallel by head - one head per core. All TensorEngine
matmuls run bf16 inputs with f32 PSUM accumulation.

Pipeline design (v3):
  - Per-batch AllToAll for batches 0-2; batch 3 is split into two
    half-batch AllToAlls so the tail only waits for a 196KB collective
    plus a 36-matmul projection.
  - Projection for batch b runs as PE "filler" matmuls inside batch b+1's
    attention slots (qt=2,3); batch-3 halves project at the tail.
  - exp runs on [128,1024] PSUM tiles (2 banks), halving ACT instructions.
  - Explicit interleave per attention slot: [sc pair p+1][fillers][pv p]
    so the in-order Tensor queue never waits on the exp dependency.
  - x prefetch is a strict 2-steps-ahead queue over a 3-buffer pool so the
    GpSimd (SWDGE) queue never backs up behind slot-paced WAR waits -
    collective triggers on that queue fire immediately.
  - cos/sin tables bf16 (2x DVE RoPE multiplies); denominator reciprocal
    broadcast via a stride-0 DMA from a DRAM bounce (off the PE).
"""

import numpy as np
from contextlib import ExitStack

import concourse.bass as bass
from concourse import bacc
import concourse.tile as tile
from concourse import mybir
from concourse.bass_utils import run_bass_kernel_spmd

B, S, C, H, HD = 4, 2048, 768, 8, 96
T = B * S            # 8192 tokens
NCORES = 8
TSLICE = T // NCORES  # 1024 tokens per core for the projection
BSLICE = S // NCORES  # 256 tokens per (core, batch)
KC = C // 128        # 6 contraction chunks of 128
F32 = mybir.dt.float32
BF16 = mybir.dt.bfloat16

SCALE = HD ** -0.5
MULT = mybir.AluOpType.mult
ADD = mybir.AluOpType.add
EXP = mybir.ActivationFunctionType.Exp
IDENT = mybir.ActivationFunctionType.Identity

SWAPMASK = []
for i in range(16):
    SWAPMASK += [2 * i + 1, 2 * i]


def build_nc():
    nc = bacc.Bacc(None, num_devices=NCORES)

    xT = nc.declare_dram_parameter("xT", [C, T], BF16, isOutput=False)
    wqkvT = nc.declare_dram_parameter("wqkvT", [C, 3 * HD], BF16, isOutput=False)
    wprojT = nc.declare_dram_parameter("wprojT", [C, C], BF16, isOutput=False)
    cosT = nc.declare_dram_parameter("cosT", [HD, S], BF16, isOutput=False)
    sT = nc.declare_dram_parameter("sT", [HD, S], BF16, isOutput=False)
    biasd = nc.declare_dram_parameter("bias", [128, KC], F32, isOutput=False)
    outd = nc.declare_dram_parameter("out", [C, TSLICE], F32, isOutput=True)

    # batches 0,1,3: one [C, 256] AllToAll each (collectives serialize on
    # one stream at ~25-30us per op, so fewer+earlier beats many small);
    # batch 2: two [C, 128] halves so its data is ready for batch-3 slots
    a2a_in = {b: nc.dram_tensor(f"a2a_in{b}", [C, BSLICE], BF16)
              for b in (0, 1, 3)}
    a2a_out = {b: nc.dram_tensor(f"a2a_out{b}", [C, BSLICE], BF16)
               for b in (0, 1, 3)}
    a2ah_in = {h: nc.dram_tensor(f"a2ah_in2{h}", [C, 128], BF16)
               for h in range(2)}
    a2ah_out = {h: nc.dram_tensor(f"a2ah_out2{h}", [C, 128], BF16)
                for h in range(2)}
    dnb = [nc.dram_tensor(f"dnb{i}", [1, 512], F32) for i in range(2)]

    with tile.TileContext(nc, num_cores=NCORES) as tc, ExitStack() as ctx:
        const = ctx.enter_context(tc.tile_pool(name="const", bufs=1))
        xtp = ctx.enter_context(tc.tile_pool(name="xtp", bufs=3))
        ropep = ctx.enter_context(tc.tile_pool(name="ropep", bufs=3))
        Pp = ctx.enter_context(tc.tile_pool(name="Pp", bufs=3))
        nrm = ctx.enter_context(tc.tile_pool(name="nrm", bufs=3))
        rcp = ctx.enter_context(tc.tile_pool(name="rcp", bufs=3))
        yp = ctx.enter_context(tc.tile_pool(name="yp", bufs=3))
        agcp = ctx.enter_context(tc.tile_pool(name="agcp", bufs=2))

        # PSUM (8 banks): scores 2x2 + PV acc 2 + qkv/proj 2
        pssc = ctx.enter_context(tc.tile_pool(name="pssc", bufs=2, space="PSUM"))
        psacc = ctx.enter_context(tc.tile_pool(name="psacc", bufs=2, space="PSUM"))
        psqkv = ctx.enter_context(tc.tile_pool(name="psqkv", bufs=2, space="PSUM"))

        # --- constants (wq + first x tiles first so the PE starts ASAP) ---
        wq_sb = const.tile([128, KC, 3 * HD], BF16)
        nc.sync.dma_start(wq_sb, wqkvT.ap().rearrange("(kc p) n -> p kc n", p=128))
        cosT_sb = const.tile([HD, S], BF16)
        sT_sb = const.tile([HD, S], BF16)
        bias_sb = const.tile([128, KC], F32)
        wp_sb = const.tile([128, KC, C], BF16)

        # persistent ping/pong per-batch q/k (transposed) and token-major v
        qT = [const.tile([HD, S], BF16, name=f"qT{i}") for i in range(2)]
        kT = [const.tile([HD, S], BF16, name=f"kT{i}") for i in range(2)]
        vA = [const.tile([128, 16, 128], BF16, name=f"vA{i}") for i in range(2)]

        # v is produced token-major directly (x-chunk stationary, Wv moving),
        # so vA needs no DMA transposes; column 96 carries ones so PV
        # accumulates the softmax denominator, columns 97..127 stay zero
        for i in range(2):
            nc.vector.memset(vA[i][:, :, HD:HD + 1], 1.0)
            nc.vector.memset(vA[i][:, :, HD + 1:128], 0.0)

        xTv = xT.ap().rearrange("(kc p) t -> p kc t", p=128)  # [128, KC, T]

        # ---------- emission helpers ----------
        state = {"xtc": {}, "agc": {}, "a2a_out": {}}
        CONS = [(b, g) for b in range(B) for g in range(4)]  # consumption order

        def load_x_group(idx, sync=False):
            """Prefetch one 512-token x group (consumption-order index)."""
            if idx >= len(CONS):
                return
            b, g = CONS[idx]
            tok0 = b * S + g * 512
            xtc = xtp.tile([128, KC, 512], BF16, tag="xtc", name="xtc")
            eng = nc.sync if sync else nc.gpsimd
            eng.dma_start(xtc, xTv[:, :, tok0:tok0 + 512])
            state["xtc"][(b, g)] = xtc

        def qkv_fillers(b, g):
            """Closures, each emitting one PE matmul of the qkv computation
            for (batch b, 512-token group g); drains are emitted inline by
            the closure that finishes each accumulation."""
            xtc = state["xtc"].pop((b, g))
            seq = slice(g * 512, (g + 1) * 512)
            ps_tiles = {}

            def drain_qk(ps, dstT):
                raw = ropep.tile([HD, 512], BF16, tag="raw", name="raw")
                nc.vector.tensor_copy(out=raw, in_=ps[0:HD, :])
                rot = ropep.tile([HD, 512], BF16, tag="rot", name="rot")
                nc.vector.stream_shuffle(rot, raw, SWAPMASK)
                t1 = ropep.tile([HD, 512], BF16, tag="t1", name="t1")
                nc.vector.tensor_tensor(t1, raw, cosT_sb[:, seq], MULT)
                t2 = ropep.tile([HD, 512], BF16, tag="t2", name="t2")
                nc.vector.tensor_tensor(t2, rot, sT_sb[:, seq], MULT)
                nc.vector.tensor_tensor(dstT[:, seq], t1, t2, ADD)

            def mk(ti, kc):
                def emit():
                    if kc == 0:
                        ps_tiles[ti] = psqkv.tile(
                            [128, 512], F32, tag="qkv", name="qkvps")
                    ps = ps_tiles[ti]
                    nc.tensor.matmul(
                        ps[0:HD, :], wq_sb[:, kc, ti * HD:(ti + 1) * HD],
                        xtc[:, kc, :],
                        start=(kc == 0), stop=(kc == KC - 1),
                    )
                    if kc == KC - 1:
                        drain_qk(ps, (qT if ti == 0 else kT)[b % 2])
                return emit

            def mk_v(c, kc):
                # token-major v: out[tok, d] = x_chunk.T @ WvT - the
                # stationary is the x tile, so vA needs no transpose
                def emit():
                    if c == 0 and kc == 0:
                        ps_tiles["v"] = psqkv.tile(
                            [128, 4, 128], F32, tag="qkv", name="vps")
                    ps = ps_tiles["v"]
                    nc.tensor.matmul(
                        ps[:, c, 0:HD],
                        xtc[:, kc, c * 128:(c + 1) * 128],
                        wq_sb[:, kc, 2 * HD:3 * HD],
                        start=(kc == 0), stop=(kc == KC - 1),
                    )
                    if c == 3 and kc == KC - 1:
                        nc.scalar.copy(
                            out=vA[b % 2][:, 4 * g:4 * g + 4, 0:HD],
                            in_=ps[:, :, 0:HD])
                return emit

            # v first: its drain is on the critical path for the next
            # batch's PV matmuls; RoPE drains can trail
            return ([mk_v(c, kc) for c in range(4) for kc in range(KC)]
                    + [mk(ti, kc) for ti in (0, 1) for kc in range(KC)])

        def proj_fillers(key, chunks, ncols, colbase):
            """Closures for proj chunks (6 matmuls + drain each). `key`
            selects the gathered buffer; output columns [colbase,
            colbase+ncols)."""
            fillers = []
            for ko in chunks:
                def mk(ko):
                    py_ref = {}

                    def emit_mm(kc):
                        if kc == 0:
                            py_ref["py"] = psqkv.tile(
                                [128, 512], F32, tag="qkv", name="pyps")
                        py = py_ref["py"]
                        nc.tensor.matmul(
                            py[:, 0:ncols],
                            wp_sb[:, kc, ko * 128:(ko + 1) * 128],
                            state["agc"][key][:, kc, :],
                            start=(kc == 0), stop=(kc == KC - 1),
                        )
                        if kc == KC - 1:
                            y = yp.tile([128, ncols], F32, tag="y", name="y")
                            nc.scalar.activation(
                                y, py[:, 0:ncols], IDENT,
                                bias=bias_sb[:, ko:ko + 1], scale=1.0,
                            )
                            nc.sync.dma_start(
                                outd.ap()[ko * 128:(ko + 1) * 128,
                                          colbase:colbase + ncols],
                                y)
                    return [lambda kc=kc: emit_mm(kc) for kc in range(KC)]
                fillers += mk(ko)
            return fillers

        def trigger_a2a(ins, outs, key):
            nc.gpsimd.collective_compute(
                "AllToAll", mybir.AluOpType.bypass,
                replica_groups=[list(range(NCORES))],
                ins=[ins.ap().opt()],
                outs=[outs.ap().opt()],
            )
            state["a2a_out"][key] = outs

        def emit_agc(key, ncols):
            """Load the gathered buffer. Emitted well after its collective
            completes so this (GpSimd-queue) DMA's wait never delays later
            collective triggers queued behind it."""
            agc = agcp.tile([128, KC, ncols], BF16, tag=f"agc{ncols}",
                            name="agc")
            nc.gpsimd.dma_start(
                agc,
                state["a2a_out"][key].ap().rearrange("(kc p) t -> p kc t",
                                                     p=128))
            state["agc"][key] = agc

        norm_b = {"pending": None}

        def emit_norm_b():
            """Part B of the previous slot's normalize: the broadcast-mult,
            the a2a staging writes, and (on batch boundaries) the collective
            trigger + gather prefetch."""
            if norm_b["pending"] is None:
                return
            b, qt, acc, bcast = norm_b["pending"]
            norm_b["pending"] = None
            onorm = nrm.tile([HD, 512], BF16, tag="onorm", name="onorm")
            nc.vector.tensor_tensor(onorm, acc[0:HD, :], bcast, MULT)
            if b != 2:
                for half in range(2):
                    j = 2 * qt + half
                    nc.sync.dma_start(
                        a2a_in[b].ap()[j * HD:(j + 1) * HD, :],
                        onorm[:, half * 256:(half + 1) * 256])
                if qt == 3:
                    trigger_a2a(a2a_in[b], a2a_out[b], b)
            else:
                # batch 2: owner j holds tokens [j*128,(j+1)*128) per
                # half-batch so each half's collective triggers 2 slots early
                bh = qt // 2
                for c in range(4):
                    j = 4 * (qt % 2) + c
                    nc.sync.dma_start(
                        a2ah_in[bh].ap()[j * HD:(j + 1) * HD, :],
                        onorm[:, c * 128:(c + 1) * 128])
                if qt % 2 == 1:
                    trigger_a2a(a2ah_in[bh], a2ah_out[bh], f"2{bh}")

        def attention_slot(b, qt, fillers):
            """One attention tile (512 q tokens, 16 k tiles as 8 pairs) with
            filler matmuls interleaved so the PE stays dense."""
            q_b, k_b, v_b = qT[b % 2], kT[b % 2], vA[b % 2]
            fill = list(fillers)
            fi = 0
            nfill = len(fill)
            acc = psacc.tile([128, 512], F32, name="acc")
            Pt_t = [None] * 8

            def emit_sc(p):
                sc = pssc.tile([128, 1024], F32, tag="sc", name="sc")
                for h in range(2):
                    kt = 2 * p + h
                    nc.tensor.matmul(
                        sc[:, h * 512:(h + 1) * 512],
                        k_b[:, kt * 128:(kt + 1) * 128],
                        q_b[:, qt * 512:(qt + 1) * 512],
                        start=True, stop=True,
                    )
                Pt = Pp.tile([128, 1024], BF16, tag="P", name="Pt")
                Pt_t[p] = Pt
                nc.scalar.activation(Pt, sc, EXP, scale=SCALE)

            def emit_pv(p):
                Pt = Pt_t[p]
                for h in range(2):
                    kt = 2 * p + h
                    nc.tensor.matmul(
                        acc, v_b[:, kt, :], Pt[:, h * 512:(h + 1) * 512],
                        start=(kt == 0), stop=(kt == 15),
                    )

            # interleave: sc(p+1) ... fillers ... pv(p)
            emit_sc(0)
            for p in range(8):
                if p + 1 < 8:
                    emit_sc(p + 1)
                # spread fillers evenly over the 8 pair slots
                ntake = (nfill * (p + 1)) // 8 - fi
                for _ in range(ntake):
                    fill[fi]()
                    fi += 1
                if p == 2:
                    # previous slot's normalize part B: by now its broadcast
                    # DMA has landed, so the DVE queue won't block on it
                    emit_norm_b()
                emit_pv(p)

            # normalize part A: denominators -> reciprocal -> DRAM bounce ->
            # stride-0 broadcast (no PE involved)
            dnrow = rcp.tile([1, 512], F32, tag="dnrow", name="dnrow")
            nc.vector.tensor_copy(out=dnrow, in_=acc[HD:HD + 1, :])
            dn = rcp.tile([128, 4], F32, tag="dn", name="dn")
            nc.sync.dma_start(dn, dnrow)
            rc = rcp.tile([128, 4], F32, tag="rc", name="rc")
            nc.vector.reciprocal(rc, dn)
            bounce = dnb[qt % 2]
            nc.sync.dma_start(bounce.ap(), rc)
            bcast = nrm.tile([HD, 512], F32, tag="bcast", name="bcast")
            b_ap = bounce.ap()
            bcast_src = bass.AP(
                tensor=b_ap.tensor, offset=b_ap.offset,
                ap=[[0, HD]] + list(b_ap.ap)[1:],
            )
            nc.sync.dma_start(bcast, bcast_src)
            norm_b["pending"] = (b, qt, acc, bcast)

        # ---------- main schedule ----------
        # x prefetch: strict consumption-order queue, 2 steps ahead.
        # The first two x tiles go right after wq on the Sync queue; the
        # RoPE tables / proj weights load behind them (not needed as early).
        load_x_group(0, sync=True)
        load_x_group(1, sync=True)
        nc.sync.dma_start(cosT_sb, cosT.ap())
        nc.sync.dma_start(sT_sb, sT.ap())
        nc.sync.dma_start(bias_sb, biasd.ap())
        nc.sync.dma_start(wp_sb, wprojT.ap().rearrange("(kc p) n -> p kc n", p=128))
        # prologue: qkv(0) dense
        for g in range(4):
            for f in qkv_fillers(0, g):
                f()
            load_x_group(g + 2)

        # proj(b) runs ~2 batches after b so even a slow collective (entry-
        # barrier skew on the first one) is done before its matmuls hit the
        # in-order Tensor queue; agc loads are placed only after their
        # collective is surely complete so the GpSimd queue never blocks
        PROJ_AT = {
            (2, 2): (0, [0, 1, 2], BSLICE, 0),
            (2, 3): (0, [3, 4, 5], BSLICE, 0),
            (3, 0): (1, [0, 1, 2], BSLICE, BSLICE),
            (3, 1): (1, [3, 4, 5], BSLICE, BSLICE),
            (3, 2): ("20", [0, 1, 2, 3, 4, 5], 128, 512),
            (3, 3): ("21", [0, 1, 2, 3, 4, 5], 128, 640),
        }
        AGC_AT = {(2, 0): (0, BSLICE), (2, 2): (1, BSLICE),
                  (3, 1): ("20", 128), (3, 2): ("21", 128)}
        for b in range(B):
            for qt in range(4):
                step = 4 + b * 4 + qt
                fillers = []
                if b + 1 < B:
                    fillers += qkv_fillers(b + 1, qt)
                if (b, qt) in PROJ_AT:
                    fillers += proj_fillers(*PROJ_AT[(b, qt)])
                attention_slot(b, qt, fillers)
                load_x_group(step + 2)
                if (b, qt) in AGC_AT:
                    emit_agc(*AGC_AT[(b, qt)])
        emit_norm_b()
        # tail: batch 3's collective + projection
        emit_agc(3, BSLICE)
        for f in proj_fillers(3, [0, 1, 2, 3, 4, 5], BSLICE, 3 * BSLICE):
            f()

    nc.compile()
    return nc


_NC_CACHE = None


def _get_nc():
    global _NC_CACHE
    if _NC_CACHE is None:
        _NC_CACHE = build_nc()
    return _NC_CACHE


def make_in_maps(x, cos, sin, Wqkv, Wproj, bproj):
    import ml_dtypes

    bf16 = ml_dtypes.bfloat16
    x = np.asarray(x, np.float32)
    cos = np.asarray(cos, np.float32)
    sin = np.asarray(sin, np.float32)
    Wqkv = np.asarray(Wqkv, np.float32)
    Wproj = np.asarray(Wproj, np.float32)
    bproj = np.asarray(bproj, np.float32)

    xT = np.ascontiguousarray(x.reshape(T, C).T.astype(bf16))  # [C, T] bf16
    wprojT = np.ascontiguousarray(Wproj.T.astype(bf16))        # [C_in, C_out]
    s = sin.copy()
    s[:, 0::2] = -sin[:, 0::2]
    cosT = np.ascontiguousarray(cos.T.astype(bf16))            # [HD, S] bf16
    sT = np.ascontiguousarray(s.T.astype(bf16))                # [HD, S] bf16
    bias2 = np.ascontiguousarray(bproj.reshape(KC, 128).T)     # [128, KC]

    in_maps = []
    for h in range(NCORES):
        wh = np.concatenate(
            [
                Wqkv[h * HD:(h + 1) * HD],                 # q rows
                Wqkv[C + h * HD:C + (h + 1) * HD],         # k rows
                Wqkv[2 * C + h * HD:2 * C + (h + 1) * HD], # v rows
            ],
            axis=0,
        )                                                  # [3*HD, C]
        wqkvT_h = np.ascontiguousarray(wh.T.astype(bf16))  # [C, 3*HD]
        in_maps.append({
            "xT": xT,
            "wqkvT": wqkvT_h,
            "wprojT": wprojT,
            "cosT": cosT,
            "sT": sT,
            "bias": bias2,
        })
    return in_maps


def assemble_output(results):
    # batches 0,1,3: core h's out cols b*256+t <-> token b*S + h*256 + t
    # batch 2: cols 512 + half*128 + t <-> token 2*S + half*1024 + h*128 + t
    y = np.empty((T, C), np.float32)
    for h in range(NCORES):
        o = results[h]["out"].T  # [1024, C]
        for b in (0, 1, 3):
            y[b * S + h * BSLICE:b * S + (h + 1) * BSLICE] = \
                o[b * BSLICE:(b + 1) * BSLICE]
        for hf in range(2):
            t0 = 2 * S + hf * 1024 + h * 128
            c0 = 2 * BSLICE + hf * 128
            y[t0:t0 + 128] = o[c0:c0 + 128]
    return y.reshape(B, S, C)


def kernel(x, cos, sin, Wqkv, Wproj, bproj, _trace=False, **run_kwargs):
    nc = _get_nc()
    in_maps = make_in_maps(x, cos, sin, Wqkv, Wproj, bproj)
    res = run_bass_kernel_spmd(
        nc, in_maps, core_ids=list(range(NCORES)), trace=_trace, **run_kwargs
    )
    out = assemble_output(res.results)
    kernel.last_results = res
    return out


if __name__ == "__main__":
    nc = build_nc()
    print("built OK, instructions:", len(nc.inst_map))


# revision 19
# speedup vs baseline: 1.0140x; 1.0140x over previous
"""Multi-head attention (B=4, S=2048, C=768, H=8, HD=96) on 8 TRN2 NeuronCores.

Strategy: tensor-parallel by head - one head per core. All TensorEngine
matmuls run bf16 inputs with f32 PSUM accumulation.

Pipeline design (v3):
  - Per-batch AllToAll for batches 0-2; batch 3 is split into two
    half-batch AllToAlls so the tail only waits for a 196KB collective
    plus a 36-matmul projection.
  - Projection for batch b runs as PE "filler" matmuls inside batch b+1's
    attention slots (qt=2,3); batch-3 halves project at the tail.
  - exp runs on [128,1024] PSUM tiles (2 banks), halving ACT instructions.
  - Explicit interleave per attention slot: [sc pair p+1][fillers][pv p]
    so the in-order Tensor queue never waits on the exp dependency.
  - x prefetch is a strict 2-steps-ahead queue over a 3-buffer pool so the
    GpSimd (SWDGE) queue never backs up behind slot-paced WAR waits -
    collective triggers on that queue fire immediately.
  - cos/sin tables bf16 (2x DVE RoPE multiplies); denominator reciprocal
    broadcast via a stride-0 DMA from a DRAM bounce (off the PE).
"""

import numpy as np
from contextlib import ExitStack

import concourse.bass as bass
from concourse import bacc
import concourse.tile as tile
from concourse import mybir
from concourse.bass_utils import run_bass_kernel_spmd

B, S, C, H, HD = 4, 2048, 768, 8, 96
T = B * S            # 8192 tokens
NCORES = 8
TSLICE = T // NCORES  # 1024 tokens per core for the projection
BSLICE = S // NCORES  # 256 tokens per (core, batch)
KC = C // 128        # 6 contraction chunks of 128
F32 = mybir.dt.float32
BF16 = mybir.dt.bfloat16

SCALE = HD ** -0.5
MULT = mybir.AluOpType.mult
ADD = mybir.AluOpType.add
EXP = mybir.ActivationFunctionType.Exp
IDENT = mybir.ActivationFunctionType.Identity

SWAPMASK = []
for i in range(16):
    SWAPMASK += [2 * i + 1, 2 * i]


def build_nc():
    nc = bacc.Bacc(None, num_devices=NCORES)

    xT = nc.declare_dram_parameter("xT", [C, T], BF16, isOutput=False)
    wqkvT = nc.declare_dram_parameter("wqkvT", [C, 3 * HD], BF16, isOutput=False)
    wprojT = nc.declare_dram_parameter("wprojT", [C, C], BF16, isOutput=False)
    cosT = nc.declare_dram_parameter("cosT", [HD, S], BF16, isOutput=False)
    sT = nc.declare_dram_parameter("sT", [HD, S], BF16, isOutput=False)
    biasd = nc.declare_dram_parameter("bias", [128, KC], F32, isOutput=False)
    outd = nc.declare_dram_parameter("out", [C, TSLICE], F32, isOutput=True)

    # batches 0,1,3: one [C, 256] AllToAll each (collectives serialize on
    # one stream at ~25-30us per op, so fewer+earlier beats many small);
    # batch 2: two [C, 128] halves so its data is ready for batch-3 slots
    a2a_in = {b: nc.dram_tensor(f"a2a_in{b}", [C, BSLICE], BF16)
              for b in (0, 1, 3)}
    a2a_out = {b: nc.dram_tensor(f"a2a_out{b}", [C, BSLICE], BF16)
               for b in (0, 1, 3)}
    a2ah_in = {h: nc.dram_tensor(f"a2ah_in2{h}", [C, 128], BF16)
               for h in range(2)}
    a2ah_out = {h: nc.dram_tensor(f"a2ah_out2{h}", [C, 128], BF16)
                for h in range(2)}
    dnb = [nc.dram_tensor(f"dnb{i}", [1, 512], F32) for i in range(2)]

    with tile.TileContext(nc, num_cores=NCORES) as tc, ExitStack() as ctx:
        const = ctx.enter_context(tc.tile_pool(name="const", bufs=1))
        xtp = ctx.enter_context(tc.tile_pool(name="xtp", bufs=5))
        ropep = ctx.enter_context(tc.tile_pool(name="ropep", bufs=3))
        Pp = ctx.enter_context(tc.tile_pool(name="Pp", bufs=3))
        nrm = ctx.enter_context(tc.tile_pool(name="nrm", bufs=3))
        rcp = ctx.enter_context(tc.tile_pool(name="rcp", bufs=3))
        yp = ctx.enter_context(tc.tile_pool(name="yp", bufs=3))
        agcp = ctx.enter_context(tc.tile_pool(name="agcp", bufs=2))

        # PSUM (8 banks): scores 2x2 + PV acc 2 + qkv/proj 2
        pssc = ctx.enter_context(tc.tile_pool(name="pssc", bufs=2, space="PSUM"))
        psacc = ctx.enter_context(tc.tile_pool(name="psacc", bufs=2, space="PSUM"))
        psqkv = ctx.enter_context(tc.tile_pool(name="psqkv", bufs=2, space="PSUM"))

        # --- constants (wq + first x tiles first so the PE starts ASAP) ---
        wq_sb = const.tile([128, KC, 3 * HD], BF16)
        nc.sync.dma_start(wq_sb, wqkvT.ap().rearrange("(kc p) n -> p kc n", p=128))
        cosT_sb = const.tile([HD, S], BF16)
        sT_sb = const.tile([HD, S], BF16)
        bias_sb = const.tile([128, KC], F32)
        wp_sb = const.tile([128, KC, C], BF16)

        # persistent ping/pong per-batch q/k (transposed) and token-major v
        qT = [const.tile([HD, S], BF16, name=f"qT{i}") for i in range(2)]
        kT = [const.tile([HD, S], BF16, name=f"kT{i}") for i in range(2)]
        vA = [const.tile([128, 16, 128], BF16, name=f"vA{i}") for i in range(2)]

        # v is produced token-major directly (x-chunk stationary, Wv moving),
        # so vA needs no DMA transposes; column 96 carries ones so PV
        # accumulates the softmax denominator, columns 97..127 stay zero
        for i in range(2):
            nc.vector.memset(vA[i][:, :, HD:HD + 1], 1.0)
            nc.vector.memset(vA[i][:, :, HD + 1:128], 0.0)

        xTv = xT.ap().rearrange("(kc p) t -> p kc t", p=128)  # [128, KC, T]

        # ---------- emission helpers ----------
        state = {"xtc": {}, "agc": {}, "a2a_out": {}}
        CONS = [(b, g) for b in range(B) for g in range(4)]  # consumption order

        def load_x_group(idx, sync=False):
            """Prefetch one 512-token x group (consumption-order index)."""
            if idx >= len(CONS):
                return
            b, g = CONS[idx]
            tok0 = b * S + g * 512
            xtc = xtp.tile([128, KC, 512], BF16, tag="xtc", name="xtc")
            eng = nc.sync if sync else nc.gpsimd
            eng.dma_start(xtc, xTv[:, :, tok0:tok0 + 512])
            state["xtc"][(b, g)] = xtc

        def qkv_fillers(b, g):
            """Closures, each emitting one PE matmul of the qkv computation
            for (batch b, 512-token group g); drains are emitted inline by
            the closure that finishes each accumulation."""
            xtc = state["xtc"].pop((b, g))
            seq = slice(g * 512, (g + 1) * 512)
            ps_tiles = {}

            def drain_qk(ps, dstT):
                raw = ropep.tile([HD, 512], BF16, tag="raw", name="raw")
                nc.vector.tensor_copy(out=raw, in_=ps[0:HD, :])
                rot = ropep.tile([HD, 512], BF16, tag="rot", name="rot")
                nc.vector.stream_shuffle(rot, raw, SWAPMASK)
                t1 = ropep.tile([HD, 512], BF16, tag="t1", name="t1")
                nc.vector.tensor_tensor(t1, raw, cosT_sb[:, seq], MULT)
                t2 = ropep.tile([HD, 512], BF16, tag="t2", name="t2")
                nc.vector.tensor_tensor(t2, rot, sT_sb[:, seq], MULT)
                nc.vector.tensor_tensor(dstT[:, seq], t1, t2, ADD)

            def mk(ti, kc):
                def emit():
                    if kc == 0:
                        ps_tiles[ti] = psqkv.tile(
                            [128, 512], F32, tag="qkv", name="qkvps")
                    ps = ps_tiles[ti]
                    nc.tensor.matmul(
                        ps[0:HD, :], wq_sb[:, kc, ti * HD:(ti + 1) * HD],
                        xtc[:, kc, :],
                        start=(kc == 0), stop=(kc == KC - 1),
                    )
                    if kc == KC - 1:
                        drain_qk(ps, (qT if ti == 0 else kT)[b % 2])
                return emit

            def mk_v(c, kc):
                # token-major v: out[tok, d] = x_chunk.T @ WvT - the
                # stationary is the x tile, so vA needs no transpose
                def emit():
                    if c == 0 and kc == 0:
                        ps_tiles["v"] = psqkv.tile(
                            [128, 4, 128], F32, tag="qkv", name="vps")
                    ps = ps_tiles["v"]
                    nc.tensor.matmul(
                        ps[:, c, 0:HD],
                        xtc[:, kc, c * 128:(c + 1) * 128],
                        wq_sb[:, kc, 2 * HD:3 * HD],
                        start=(kc == 0), stop=(kc == KC - 1),
                    )
                    if c == 3 and kc == KC - 1:
                        nc.scalar.copy(
                            out=vA[b % 2][:, 4 * g:4 * g + 4, 0:HD],
                            in_=ps[:, :, 0:HD])
                return emit

            # v first: its drain is on the critical path for the next
            # batch's PV matmuls; RoPE drains can trail
            return ([mk_v(c, kc) for c in range(4) for kc in range(KC)]
                    + [mk(ti, kc) for ti in (0, 1) for kc in range(KC)])

        def proj_fillers(key, chunks, ncols, colbase):
            """Closures for proj chunks (6 matmuls + drain each). `key`
            selects the gathered buffer; output columns [colbase,
            colbase+ncols)."""
            fillers = []
            for ko in chunks:
                def mk(ko):
                    py_ref = {}

                    def emit_mm(kc):
                        if kc == 0:
                            py_ref["py"] = psqkv.tile(
                                [128, 512], F32, tag="qkv", name="pyps")
                        py = py_ref["py"]
                        nc.tensor.matmul(
                            py[:, 0:ncols],
                            wp_sb[:, kc, ko * 128:(ko + 1) * 128],
                            state["agc"][key][:, kc, :],
                            start=(kc == 0), stop=(kc == KC - 1),
                        )
                        if kc == KC - 1:
                            y = yp.tile([128, ncols], F32, tag="y", name="y")
                            nc.scalar.activation(
                                y, py[:, 0:ncols], IDENT,
                                bias=bias_sb[:, ko:ko + 1], scale=1.0,
                            )
                            nc.gpsimd.dma_start(
                                outd.ap()[ko * 128:(ko + 1) * 128,
                                          colbase:colbase + ncols],
                                y)
                    return [lambda kc=kc: emit_mm(kc) for kc in range(KC)]
                fillers += mk(ko)
            return fillers

        def trigger_a2a(ins, outs, key):
            nc.gpsimd.collective_compute(
                "AllToAll", mybir.AluOpType.bypass,
                replica_groups=[list(range(NCORES))],
                ins=[ins.ap().opt()],
                outs=[outs.ap().opt()],
            )
            state["a2a_out"][key] = outs

        def emit_agc(key, ncols):
            """Load the gathered buffer. Emitted well after its collective
            completes so this (GpSimd-queue) DMA's wait never delays later
            collective triggers queued behind it."""
            agc = agcp.tile([128, KC, ncols], BF16, tag=f"agc{ncols}",
                            name="agc")
            nc.gpsimd.dma_start(
                agc,
                state["a2a_out"][key].ap().rearrange("(kc p) t -> p kc t",
                                                     p=128))
            state["agc"][key] = agc

        norm_b = {"pending": None}

        def emit_norm_b():
            """Part B of the previous slot's normalize: the broadcast-mult,
            the a2a staging writes, and (on batch boundaries) the collective
            trigger + gather prefetch."""
            if norm_b["pending"] is None:
                return
            b, qt, acc, bcast = norm_b["pending"]
            norm_b["pending"] = None
            onorm = nrm.tile([HD, 512], BF16, tag="onorm", name="onorm")
            nc.vector.tensor_tensor(onorm, acc[0:HD, :], bcast, MULT)
            if b != 2:
                for half in range(2):
                    j = 2 * qt + half
                    nc.sync.dma_start(
                        a2a_in[b].ap()[j * HD:(j + 1) * HD, :],
                        onorm[:, half * 256:(half + 1) * 256])
                if qt == 3:
                    trigger_a2a(a2a_in[b], a2a_out[b], b)
            else:
                # batch 2: owner j holds tokens [j*128,(j+1)*128) per
                # half-batch so each half's collective triggers 2 slots early
                bh = qt // 2
                for c in range(4):
                    j = 4 * (qt % 2) + c
                    nc.sync.dma_start(
                        a2ah_in[bh].ap()[j * HD:(j + 1) * HD, :],
                        onorm[:, c * 128:(c + 1) * 128])
                if qt % 2 == 1:
                    trigger_a2a(a2ah_in[bh], a2ah_out[bh], f"2{bh}")

        def attention_slot(b, qt, fillers, late=False):
            """One attention tile (512 q tokens, 16 k tiles as 8 pairs) with
            filler matmuls interleaved so the PE stays dense. late=True
            packs fillers into the second half of the slot (for proj chunks
            whose gathered buffer lands mid-slot)."""
            q_b, k_b, v_b = qT[b % 2], kT[b % 2], vA[b % 2]
            fill = list(fillers)
            fi = 0
            nfill = len(fill)
            acc = psacc.tile([128, 512], F32, name="acc")
            Pt_t = [None] * 8

            def emit_sc(p):
                sc = pssc.tile([128, 1024], F32, tag="sc", name="sc")
                for h in range(2):
                    kt = 2 * p + h
                    nc.tensor.matmul(
                        sc[:, h * 512:(h + 1) * 512],
                        k_b[:, kt * 128:(kt + 1) * 128],
                        q_b[:, qt * 512:(qt + 1) * 512],
                        start=True, stop=True,
                    )
                Pt = Pp.tile([128, 1024], BF16, tag="P", name="Pt")
                Pt_t[p] = Pt
                nc.scalar.activation(Pt, sc, EXP, scale=SCALE)

            def emit_pv(p):
                Pt = Pt_t[p]
                for h in range(2):
                    kt = 2 * p + h
                    nc.tensor.matmul(
                        acc, v_b[:, kt, :], Pt[:, h * 512:(h + 1) * 512],
                        start=(kt == 0), stop=(kt == 15),
                    )

            # interleave: sc(p+1) ... fillers ... pv(p)
            emit_sc(0)
            for p in range(8):
                if p + 1 < 8:
                    emit_sc(p + 1)
                # spread fillers over the pair slots
                if late:
                    ntake = (nfill * max(0, p - 2)) // 5 - fi
                else:
                    ntake = (nfill * (p + 1)) // 8 - fi
                for _ in range(ntake):
                    fill[fi]()
                    fi += 1
                if p == 2:
                    # previous slot's normalize part B: by now its broadcast
                    # DMA has landed, so the DVE queue won't block on it
                    emit_norm_b()
                emit_pv(p)

            # normalize part A: denominators -> reciprocal -> DRAM bounce ->
            # stride-0 broadcast (no PE involved)
            dnrow = rcp.tile([1, 512], F32, tag="dnrow", name="dnrow")
            nc.vector.tensor_copy(out=dnrow, in_=acc[HD:HD + 1, :])
            dn = rcp.tile([128, 4], F32, tag="dn", name="dn")
            nc.sync.dma_start(dn, dnrow)
            rc = rcp.tile([128, 4], F32, tag="rc", name="rc")
            nc.vector.reciprocal(rc, dn)
            bounce = dnb[qt % 2]
            nc.sync.dma_start(bounce.ap(), rc)
            bcast = nrm.tile([HD, 512], F32, tag="bcast", name="bcast")
            b_ap = bounce.ap()
            bcast_src = bass.AP(
                tensor=b_ap.tensor, offset=b_ap.offset,
                ap=[[0, HD]] + list(b_ap.ap)[1:],
            )
            nc.sync.dma_start(bcast, bcast_src)
            norm_b["pending"] = (b, qt, acc, bcast)

        # ---------- main schedule ----------
        # x prefetch: strict consumption-order queue, 2 steps ahead.
        # The first two x tiles go right after wq on the Sync queue; the
        # RoPE tables / proj weights load behind them (not needed as early).
        load_x_group(0, sync=True)
        load_x_group(1, sync=True)
        nc.sync.dma_start(cosT_sb, cosT.ap())
        nc.sync.dma_start(sT_sb, sT.ap())
        nc.sync.dma_start(bias_sb, biasd.ap())
        nc.sync.dma_start(wp_sb, wprojT.ap().rearrange("(kc p) n -> p kc n", p=128))
        # prologue: qkv(0) dense
        for g in range(4):
            for f in qkv_fillers(0, g):
                f()
            load_x_group(g + 2)

        # proj(b) runs ~2 batches after b so even a slow collective (entry-
        # barrier skew on the first one) is done before its matmuls hit the
        # in-order Tensor queue; agc loads are placed only after their
        # collective is surely complete so the GpSimd queue never blocks
        PROJ_AT = {
            (2, 2): (0, [0, 1, 2], BSLICE, 0),
            (2, 3): (0, [3, 4, 5], BSLICE, 0),
            (3, 0): (1, [0, 1, 2], BSLICE, BSLICE),
            (3, 1): (1, [3, 4, 5], BSLICE, BSLICE),
            (3, 2): ("20", [0, 1, 2, 3, 4, 5], 128, 512),
            (3, 3): ("21", [0, 1, 2, 3, 4, 5], 128, 640),
        }
        AGC_AT = {(2, 0): (0, BSLICE), (2, 2): (1, BSLICE),
                  (3, 1): ("20", 128), (3, 2): ("21", 128)}
        for b in range(B):
            for qt in range(4):
                step = 4 + b * 4 + qt
                fillers = []
                if b + 1 < B:
                    fillers += qkv_fillers(b + 1, qt)
                if (b, qt) in PROJ_AT:
                    fillers += proj_fillers(*PROJ_AT[(b, qt)])
                attention_slot(b, qt, fillers, late=((b, qt) == (3, 3)))
                load_x_group(step + 2)
                if (b, qt) in AGC_AT:
                    emit_agc(*AGC_AT[(b, qt)])
        emit_norm_b()
        # tail: batch 3's collective + projection
        emit_agc(3, BSLICE)
        for f in proj_fillers(3, [0, 1, 2, 3, 4, 5], BSLICE, 3 * BSLICE):
            f()

    nc.compile()
    return nc


_NC_CACHE = None


def _get_nc():
    global _NC_CACHE
    if _NC_CACHE is None:
        _NC_CACHE = build_nc()
    return _NC_CACHE


def make_in_maps(x, cos, sin, Wqkv, Wproj, bproj):
    import ml_dtypes

    bf16 = ml_dtypes.bfloat16
    x = np.asarray(x, np.float32)
    cos = np.asarray(cos, np.float32)
    sin = np.asarray(sin, np.float32)
    Wqkv = np.asarray(Wqkv, np.float32)
    Wproj = np.asarray(Wproj, np.float32)
    bproj = np.asarray(bproj, np.float32)

    xT = np.ascontiguousarray(x.reshape(T, C).T.astype(bf16))  # [C, T] bf16
    wprojT = np.ascontiguousarray(Wproj.T.astype(bf16))        # [C_in, C_out]
    s = sin.copy()
    s[:, 0::2] = -sin[:, 0::2]
    cosT = np.ascontiguousarray(cos.T.astype(bf16))            # [HD, S] bf16
    sT = np.ascontiguousarray(s.T.astype(bf16))                # [HD, S] bf16
    bias2 = np.ascontiguousarray(bproj.reshape(KC, 128).T)     # [128, KC]

    in_maps = []
    for h in range(NCORES):
        wh = np.concatenate(
            [
                Wqkv[h * HD:(h + 1) * HD],                 # q rows
                Wqkv[C + h * HD:C + (h + 1) * HD],         # k rows
                Wqkv[2 * C + h * HD:2 * C + (h + 1) * HD], # v rows
            ],
            axis=0,
        )                                                  # [3*HD, C]
        wqkvT_h = np.ascontiguousarray(wh.T.astype(bf16))  # [C, 3*HD]
        in_maps.append({
            "xT": xT,
            "wqkvT": wqkvT_h,
            "wprojT": wprojT,
            "cosT": cosT,
            "sT": sT,
            "bias": bias2,
        })
    return in_maps


def assemble_output(results):
    # batches 0,1,3: core h's out cols b*256+t <-> token b*S + h*256 + t
    # batch 2: cols 512 + half*128 + t <-> token 2*S + half*1024 + h*128 + t
    y = np.empty((T, C), np.float32)
    for h in range(NCORES):
        o = results[h]["out"].T  # [1024, C]
        for b in (0, 1, 3):
            y[b * S + h * BSLICE:b * S + (h + 1) * BSLICE] = \
                o[b * BSLICE:(b + 1) * BSLICE]
        for hf in range(2):
            t0 = 2 * S + hf * 1024 + h * 128
            c0 = 2 * BSLICE + hf * 128
            y[t0:t0 + 128] = o[c0:c0 + 128]
    return y.reshape(B, S, C)


def kernel(x, cos, sin, Wqkv, Wproj, bproj, _trace=False, **run_kwargs):
    nc = _get_nc()
    in_maps = make_in_maps(x, cos, sin, Wqkv, Wproj, bproj)
    res = run_bass_kernel_spmd(
        nc, in_maps, core_ids=list(range(NCORES)), trace=_trace, **run_kwargs
    )
    out = assemble_output(res.results)
    kernel.last_results = res
    return out


if __name__ == "__main__":
    nc = build_nc()
    print("built OK, instructions:", len(nc.inst_map))


# revision 20
# speedup vs baseline: 1.0250x; 1.0109x over previous
"""Multi-head attention (B=4, S=2048, C=768, H=8, HD=96) on 8 TRN2 NeuronCores.

Strategy: tensor-parallel by head - one head per core. All TensorEngine
matmuls run bf16 inputs with f32 PSUM accumulation.

Pipeline design (v9, 463us baseline -> ~318us):
  - v is produced token-major directly on the PE (x-chunk stationary, Wv
    moving, 24 N=96 matmuls/group) - no DMA transposes. This matters
    beyond the transposes themselves: DMA_TRANSPOSE serializes against
    SWDGE DMAs via Tile-inserted Sync-queue barriers that were blocking
    the whole DMA pipeline ~7us per slot.
  - Collectives serialize on one stream at ~20-35us per op regardless of
    size, so: one AllToAll per batch (0,1,3) triggered as early as its
    staging allows, with batch 2 split into two half-batch ops purely so
    its gathers land in time for batch-3 slots. Projections run as PE
    "filler" matmuls ~2 batches after their own batch; batch 3 projects
    at the tail behind the last collective.
  - Gather (agc) loads are emitted only after their collective is surely
    complete, so the GpSimd queue (x prefetch + triggers) never blocks.
  - exp runs on [128,1024] PSUM tiles (2 banks), halving ACT instructions
    so ACT never paces the PE.
  - Explicit interleave per attention slot: [sc pair p+1][fillers][pv p]
    so the in-order Tensor queue never waits on the exp dependency and
    the HAM clock-gate stays warm.
  - x prefetch is a strict consumption-order queue, 2 steps ahead over a
    5-buffer pool (no WAR waits on the GpSimd queue).
  - cos/sin tables bf16 (2x DVE RoPE multiplies); denominator reciprocal
    broadcast via a stride-0 DMA from a DRAM bounce (off the PE).
"""

import numpy as np
from contextlib import ExitStack

import concourse.bass as bass
from concourse import bacc
import concourse.tile as tile
from concourse import mybir
from concourse.bass_utils import run_bass_kernel_spmd

B, S, C, H, HD = 4, 2048, 768, 8, 96
T = B * S            # 8192 tokens
NCORES = 8
TSLICE = T // NCORES  # 1024 tokens per core for the projection
BSLICE = S // NCORES  # 256 tokens per (core, batch)
KC = C // 128        # 6 contraction chunks of 128
F32 = mybir.dt.float32
BF16 = mybir.dt.bfloat16

SCALE = HD ** -0.5
MULT = mybir.AluOpType.mult
ADD = mybir.AluOpType.add
EXP = mybir.ActivationFunctionType.Exp
IDENT = mybir.ActivationFunctionType.Identity

SWAPMASK = []
for i in range(16):
    SWAPMASK += [2 * i + 1, 2 * i]


def build_nc():
    nc = bacc.Bacc(None, num_devices=NCORES)

    xT = nc.declare_dram_parameter("xT", [C, T], BF16, isOutput=False)
    wqkvT = nc.declare_dram_parameter("wqkvT", [C, 3 * HD], BF16, isOutput=False)
    wprojT = nc.declare_dram_parameter("wprojT", [C, C], BF16, isOutput=False)
    cosT = nc.declare_dram_parameter("cosT", [HD, S], BF16, isOutput=False)
    sT = nc.declare_dram_parameter("sT", [HD, S], BF16, isOutput=False)
    biasd = nc.declare_dram_parameter("bias", [128, KC], F32, isOutput=False)
    outd = nc.declare_dram_parameter("out", [C, TSLICE], F32, isOutput=True)

    # batches 0,1,3: one [C, 256] AllToAll each (collectives serialize on
    # one stream at ~25-30us per op, so fewer+earlier beats many small);
    # batch 2: two [C, 128] halves so its data is ready for batch-3 slots
    a2a_in = {b: nc.dram_tensor(f"a2a_in{b}", [C, BSLICE], BF16)
              for b in (0, 1, 3)}
    a2a_out = {b: nc.dram_tensor(f"a2a_out{b}", [C, BSLICE], BF16)
               for b in (0, 1, 3)}
    a2ah_in = {h: nc.dram_tensor(f"a2ah_in2{h}", [C, 128], BF16)
               for h in range(2)}
    a2ah_out = {h: nc.dram_tensor(f"a2ah_out2{h}", [C, 128], BF16)
                for h in range(2)}
    dnb = [nc.dram_tensor(f"dnb{i}", [1, 512], F32) for i in range(2)]

    with tile.TileContext(nc, num_cores=NCORES) as tc, ExitStack() as ctx:
        const = ctx.enter_context(tc.tile_pool(name="const", bufs=1))
        xtp = ctx.enter_context(tc.tile_pool(name="xtp", bufs=5))
        ropep = ctx.enter_context(tc.tile_pool(name="ropep", bufs=3))
        Pp = ctx.enter_context(tc.tile_pool(name="Pp", bufs=3))
        nrm = ctx.enter_context(tc.tile_pool(name="nrm", bufs=3))
        rcp = ctx.enter_context(tc.tile_pool(name="rcp", bufs=3))
        yp = ctx.enter_context(tc.tile_pool(name="yp", bufs=3))
        agcp = ctx.enter_context(tc.tile_pool(name="agcp", bufs=2))

        # PSUM (8 banks): scores 2x2 + PV acc 2 + qkv/proj 2
        pssc = ctx.enter_context(tc.tile_pool(name="pssc", bufs=2, space="PSUM"))
        psacc = ctx.enter_context(tc.tile_pool(name="psacc", bufs=2, space="PSUM"))
        psqkv = ctx.enter_context(tc.tile_pool(name="psqkv", bufs=2, space="PSUM"))

        # --- constants (wq + first x tiles first so the PE starts ASAP) ---
        wq_sb = const.tile([128, KC, 3 * HD], BF16)
        nc.sync.dma_start(wq_sb, wqkvT.ap().rearrange("(kc p) n -> p kc n", p=128))
        cosT_sb = const.tile([HD, S], BF16)
        sT_sb = const.tile([HD, S], BF16)
        bias_sb = const.tile([128, KC], F32)
        wp_sb = const.tile([128, KC, C], BF16)

        # persistent ping/pong per-batch q/k (transposed) and token-major v
        qT = [const.tile([HD, S], BF16, name=f"qT{i}") for i in range(2)]
        kT = [const.tile([HD, S], BF16, name=f"kT{i}") for i in range(2)]
        vA = [const.tile([128, 16, 128], BF16, name=f"vA{i}") for i in range(2)]

        # v is produced token-major directly (x-chunk stationary, Wv moving),
        # so vA needs no DMA transposes; column 96 carries ones so PV
        # accumulates the softmax denominator, columns 97..127 stay zero
        for i in range(2):
            nc.vector.memset(vA[i][:, :, HD:HD + 1], 1.0)
            nc.vector.memset(vA[i][:, :, HD + 1:128], 0.0)

        xTv = xT.ap().rearrange("(kc p) t -> p kc t", p=128)  # [128, KC, T]

        # ---------- emission helpers ----------
        state = {"xtc": {}, "agc": {}, "a2a_out": {}}
        CONS = [(b, g) for b in range(B) for g in range(4)]  # consumption order

        def load_x_group(idx, sync=False):
            """Prefetch one 512-token x group (consumption-order index)."""
            if idx >= len(CONS):
                return
            b, g = CONS[idx]
            tok0 = b * S + g * 512
            xtc = xtp.tile([128, KC, 512], BF16, tag="xtc", name="xtc")
            eng = nc.sync if sync else nc.gpsimd
            eng.dma_start(xtc, xTv[:, :, tok0:tok0 + 512])
            state["xtc"][(b, g)] = xtc

        def qkv_fillers(b, g):
            """Closures, each emitting one PE matmul of the qkv computation
            for (batch b, 512-token group g); drains are emitted inline by
            the closure that finishes each accumulation."""
            xtc = state["xtc"].pop((b, g))
            seq = slice(g * 512, (g + 1) * 512)
            ps_tiles = {}

            def drain_qk(ps, dstT):
                raw = ropep.tile([HD, 512], BF16, tag="raw", name="raw")
                nc.vector.tensor_copy(out=raw, in_=ps[0:HD, :])
                rot = ropep.tile([HD, 512], BF16, tag="rot", name="rot")
                nc.vector.stream_shuffle(rot, raw, SWAPMASK)
                t1 = ropep.tile([HD, 512], BF16, tag="t1", name="t1")
                nc.vector.tensor_tensor(t1, raw, cosT_sb[:, seq], MULT)
                t2 = ropep.tile([HD, 512], BF16, tag="t2", name="t2")
                nc.vector.tensor_tensor(t2, rot, sT_sb[:, seq], MULT)
                nc.vector.tensor_tensor(dstT[:, seq], t1, t2, ADD)

            def mk(ti, kc):
                def emit():
                    if kc == 0:
                        ps_tiles[ti] = psqkv.tile(
                            [128, 512], F32, tag="qkv", name="qkvps")
                    ps = ps_tiles[ti]
                    nc.tensor.matmul(
                        ps[0:HD, :], wq_sb[:, kc, ti * HD:(ti + 1) * HD],
                        xtc[:, kc, :],
                        start=(kc == 0), stop=(kc == KC - 1),
                    )
                    if kc == KC - 1:
                        drain_qk(ps, (qT if ti == 0 else kT)[b % 2])
                return emit

            def mk_v(c, kc):
                # token-major v: out[tok, d] = x_chunk.T @ WvT - the
                # stationary is the x tile, so vA needs no transpose
                def emit():
                    if c == 0 and kc == 0:
                        ps_tiles["v"] = psqkv.tile(
                            [128, 4, 128], F32, tag="qkv", name="vps")
                    ps = ps_tiles["v"]
                    nc.tensor.matmul(
                        ps[:, c, 0:HD],
                        xtc[:, kc, c * 128:(c + 1) * 128],
                        wq_sb[:, kc, 2 * HD:3 * HD],
                        start=(kc == 0), stop=(kc == KC - 1),
                    )
                    if c == 3 and kc == KC - 1:
                        nc.scalar.copy(
                            out=vA[b % 2][:, 4 * g:4 * g + 4, 0:HD],
                            in_=ps[:, :, 0:HD])
                return emit

            # v first: its drain is on the critical path for the next
            # batch's PV matmuls; RoPE drains can trail
            return ([mk_v(c, kc) for c in range(4) for kc in range(KC)]
                    + [mk(ti, kc) for ti in (0, 1) for kc in range(KC)])

        def proj_fillers(key, chunks, ncols, colbase):
            """Closures for proj chunks (6 matmuls + drain each). `key`
            selects the gathered buffer; output columns [colbase,
            colbase+ncols)."""
            fillers = []
            for ko in chunks:
                def mk(ko):
                    py_ref = {}

                    def emit_mm(kc):
                        if kc == 0:
                            py_ref["py"] = psqkv.tile(
                                [128, 512], F32, tag="qkv", name="pyps")
                        py = py_ref["py"]
                        nc.tensor.matmul(
                            py[:, 0:ncols],
                            wp_sb[:, kc, ko * 128:(ko + 1) * 128],
                            state["agc"][key][:, kc, :],
                            start=(kc == 0), stop=(kc == KC - 1),
                        )
                        if kc == KC - 1:
                            y = yp.tile([128, ncols], F32, tag="y", name="y")
                            nc.scalar.activation(
                                y, py[:, 0:ncols], IDENT,
                                bias=bias_sb[:, ko:ko + 1], scale=1.0,
                            )
                            nc.gpsimd.dma_start(
                                outd.ap()[ko * 128:(ko + 1) * 128,
                                          colbase:colbase + ncols],
                                y)
                    return [lambda kc=kc: emit_mm(kc) for kc in range(KC)]
                fillers += mk(ko)
            return fillers

        def trigger_a2a(ins, outs, key):
            nc.gpsimd.collective_compute(
                "AllToAll", mybir.AluOpType.bypass,
                replica_groups=[list(range(NCORES))],
                ins=[ins.ap().opt()],
                outs=[outs.ap().opt()],
            )
            state["a2a_out"][key] = outs

        def emit_agc(key, ncols):
            """Load the gathered buffer. Emitted well after its collective
            completes so this (GpSimd-queue) DMA's wait never delays later
            collective triggers queued behind it."""
            agc = agcp.tile([128, KC, ncols], BF16, tag=f"agc{ncols}",
                            name="agc")
            nc.gpsimd.dma_start(
                agc,
                state["a2a_out"][key].ap().rearrange("(kc p) t -> p kc t",
                                                     p=128))
            state["agc"][key] = agc

        norm_b = {"pending": None}

        def emit_norm_b():
            """Part B of the previous slot's normalize: the broadcast-mult,
            the a2a staging writes, and (on batch boundaries) the collective
            trigger + gather prefetch."""
            if norm_b["pending"] is None:
                return
            b, qt, acc, bcast = norm_b["pending"]
            norm_b["pending"] = None
            onorm = nrm.tile([HD, 512], BF16, tag="onorm", name="onorm")
            nc.vector.tensor_tensor(onorm, acc[0:HD, :], bcast, MULT)
            if b != 2:
                for half in range(2):
                    j = 2 * qt + half
                    nc.sync.dma_start(
                        a2a_in[b].ap()[j * HD:(j + 1) * HD, :],
                        onorm[:, half * 256:(half + 1) * 256])
                if qt == 3:
                    trigger_a2a(a2a_in[b], a2a_out[b], b)
            else:
                # batch 2: owner j holds tokens [j*128,(j+1)*128) per
                # half-batch so each half's collective triggers 2 slots early
                bh = qt // 2
                for c in range(4):
                    j = 4 * (qt % 2) + c
                    nc.sync.dma_start(
                        a2ah_in[bh].ap()[j * HD:(j + 1) * HD, :],
                        onorm[:, c * 128:(c + 1) * 128])
                if qt % 2 == 1:
                    trigger_a2a(a2ah_in[bh], a2ah_out[bh], f"2{bh}")

        def attention_slot(b, qt, fillers, late=False):
            """One attention tile (512 q tokens, 16 k tiles as 8 pairs) with
            filler matmuls interleaved so the PE stays dense. late=True
            packs fillers into the second half of the slot (for proj chunks
            whose gathered buffer lands mid-slot)."""
            q_b, k_b, v_b = qT[b % 2], kT[b % 2], vA[b % 2]
            fill = list(fillers)
            fi = 0
            nfill = len(fill)
            acc = psacc.tile([128, 512], F32, name="acc")
            Pt_t = [None] * 8

            def emit_sc(p):
                sc = pssc.tile([128, 1024], F32, tag="sc", name="sc")
                for h in range(2):
                    kt = 2 * p + h
                    nc.tensor.matmul(
                        sc[:, h * 512:(h + 1) * 512],
                        k_b[:, kt * 128:(kt + 1) * 128],
                        q_b[:, qt * 512:(qt + 1) * 512],
                        start=True, stop=True,
                    )
                Pt = Pp.tile([128, 1024], BF16, tag="P", name="Pt")
                Pt_t[p] = Pt
                nc.scalar.activation(Pt, sc, EXP, scale=SCALE)

            def emit_pv(p):
                Pt = Pt_t[p]
                for h in range(2):
                    kt = 2 * p + h
                    nc.tensor.matmul(
                        acc, v_b[:, kt, :], Pt[:, h * 512:(h + 1) * 512],
                        start=(kt == 0), stop=(kt == 15),
                    )

            # interleave: sc(p+1) ... fillers ... pv(p)
            emit_sc(0)
            for p in range(8):
                if p + 1 < 8:
                    emit_sc(p + 1)
                # spread fillers over the pair slots
                if late:
                    ntake = (nfill * max(0, p - 2)) // 5 - fi
                else:
                    ntake = (nfill * (p + 1)) // 8 - fi
                for _ in range(ntake):
                    fill[fi]()
                    fi += 1
                if p == 2:
                    # previous slot's normalize part B: by now its broadcast
                    # DMA has landed, so the DVE queue won't block on it
                    emit_norm_b()
                emit_pv(p)

            # normalize part A: denominators -> reciprocal -> DRAM bounce ->
            # stride-0 broadcast (no PE involved)
            dnrow = rcp.tile([1, 512], F32, tag="dnrow", name="dnrow")
            nc.vector.tensor_copy(out=dnrow, in_=acc[HD:HD + 1, :])
            dn = rcp.tile([128, 4], F32, tag="dn", name="dn")
            nc.sync.dma_start(dn, dnrow)
            rc = rcp.tile([128, 4], F32, tag="rc", name="rc")
            nc.vector.reciprocal(rc, dn)
            bounce = dnb[qt % 2]
            nc.sync.dma_start(bounce.ap(), rc)
            bcast = nrm.tile([HD, 512], F32, tag="bcast", name="bcast")
            b_ap = bounce.ap()
            bcast_src = bass.AP(
                tensor=b_ap.tensor, offset=b_ap.offset,
                ap=[[0, HD]] + list(b_ap.ap)[1:],
            )
            nc.sync.dma_start(bcast, bcast_src)
            norm_b["pending"] = (b, qt, acc, bcast)

        # ---------- main schedule ----------
        # x prefetch: strict consumption-order queue, 2 steps ahead.
        # The first two x tiles go right after wq on the Sync queue; the
        # RoPE tables / proj weights load behind them (not needed as early).
        load_x_group(0, sync=True)
        load_x_group(1, sync=True)
        nc.sync.dma_start(cosT_sb, cosT.ap())
        nc.sync.dma_start(sT_sb, sT.ap())
        nc.sync.dma_start(bias_sb, biasd.ap())
        nc.sync.dma_start(wp_sb, wprojT.ap().rearrange("(kc p) n -> p kc n", p=128))
        # prologue: qkv(0) dense
        for g in range(4):
            for f in qkv_fillers(0, g):
                f()
            load_x_group(g + 2)

        # proj(b) runs ~2 batches after b so even a slow collective (entry-
        # barrier skew on the first one) is done before its matmuls hit the
        # in-order Tensor queue; agc loads are placed only after their
        # collective is surely complete so the GpSimd queue never blocks
        PROJ_AT = {
            (2, 2): (0, [0, 1, 2], BSLICE, 0),
            (2, 3): (0, [3, 4, 5], BSLICE, 0),
            (3, 0): (1, [0, 1, 2], BSLICE, BSLICE),
            (3, 1): (1, [3, 4, 5], BSLICE, BSLICE),
            (3, 2): ("20", [0, 1, 2, 3, 4, 5], 128, 512),
            (3, 3): ("21", [0, 1, 2, 3, 4, 5], 128, 640),
        }
        AGC_AT = {(2, 0): (0, BSLICE), (2, 2): (1, BSLICE),
                  (3, 1): ("20", 128), (3, 2): ("21", 128)}
        for b in range(B):
            for qt in range(4):
                step = 4 + b * 4 + qt
                fillers = []
                if b + 1 < B:
                    fillers += qkv_fillers(b + 1, qt)
                if (b, qt) in PROJ_AT:
                    fillers += proj_fillers(*PROJ_AT[(b, qt)])
                attention_slot(b, qt, fillers, late=((b, qt) == (3, 3)))
                load_x_group(step + 2)
                if (b, qt) in AGC_AT:
                    emit_agc(*AGC_AT[(b, qt)])
        emit_norm_b()
        # tail: batch 3's collective + projection
        emit_agc(3, BSLICE)
        for f in proj_fillers(3, [0, 1, 2, 3, 4, 5], BSLICE, 3 * BSLICE):
            f()

    nc.compile()
    return nc


_NC_CACHE = None


def _get_nc():
    global _NC_CACHE
    if _NC_CACHE is None:
        _NC_CACHE = build_nc()
    return _NC_CACHE


def make_in_maps(x, cos, sin, Wqkv, Wproj, bproj):
    import ml_dtypes

    bf16 = ml_dtypes.bfloat16
    x = np.asarray(x, np.float32)
    cos = np.asarray(cos, np.float32)
    sin = np.asarray(sin, np.float32)
    Wqkv = np.asarray(Wqkv, np.float32)
    Wproj = np.asarray(Wproj, np.float32)
    bproj = np.asarray(bproj, np.float32)

    xT = np.ascontiguousarray(x.reshape(T, C).T.astype(bf16))  # [C, T] bf16
    wprojT = np.ascontiguousarray(Wproj.T.astype(bf16))        # [C_in, C_out]
    s = sin.copy()
    s[:, 0::2] = -sin[:, 0::2]
    cosT = np.ascontiguousarray(cos.T.astype(bf16))            # [HD, S] bf16
    sT = np.ascontiguousarray(s.T.astype(bf16))                # [HD, S] bf16
    bias2 = np.ascontiguousarray(bproj.reshape(KC, 128).T)     # [128, KC]

    in_maps = []
    for h in range(NCORES):
        wh = np.concatenate(
            [
                Wqkv[h * HD:(h + 1) * HD],                 # q rows
                Wqkv[C + h * HD:C + (h + 1) * HD],         # k rows
                Wqkv[2 * C + h * HD:2 * C + (h + 1) * HD], # v rows
            ],
            axis=0,
        )                                                  # [3*HD, C]
        wqkvT_h = np.ascontiguousarray(wh.T.astype(bf16))  # [C, 3*HD]
        in_maps.append({
            "xT": xT,
            "wqkvT": wqkvT_h,
            "wprojT": wprojT,
            "cosT": cosT,
            "sT": sT,
            "bias": bias2,
        })
    return in_maps


def assemble_output(results):
    # batches 0,1,3: core h's out cols b*256+t <-> token b*S + h*256 + t
    # batch 2: cols 512 + half*128 + t <-> token 2*S + half*1024 + h*128 + t
    y = np.empty((T, C), np.float32)
    for h in range(NCORES):
        o = results[h]["out"].T  # [1024, C]
        for b in (0, 1, 3):
            y[b * S + h * BSLICE:b * S + (h + 1) * BSLICE] = \
                o[b * BSLICE:(b + 1) * BSLICE]
        for hf in range(2):
            t0 = 2 * S + hf * 1024 + h * 128
            c0 = 2 * BSLICE + hf * 128
            y[t0:t0 + 128] = o[c0:c0 + 128]
    return y.reshape(B, S, C)


def kernel(x, cos, sin, Wqkv, Wproj, bproj, _trace=False, **run_kwargs):
    nc = _get_nc()
    in_maps = make_in_maps(x, cos, sin, Wqkv, Wproj, bproj)
    res = run_bass_kernel_spmd(
        nc, in_maps, core_ids=list(range(NCORES)), trace=_trace, **run_kwargs
    )
    out = assemble_output(res.results)
    kernel.last_results = res
    return out


if __name__ == "__main__":
    nc = build_nc()
    print("built OK, instructions:", len(nc.inst_map))
